# revision 10
# baseline (speedup 1.0000x reference)
"""Trainium2 Bass kernel for CRATE-style subspace attention (nn_Attention_37091337568712).

Reference computation (fp32):
    w = x @ Wqkv                    # (b, n, 1024), shared q=k=v projection
    w -> (b, h=16, n, d=64)
    S = (w @ w^T) * d^-0.5          # per head, (b, h, n, n)
    attn = softmax(S, axis=-1) * (1 - mask[:, None, None, :])
    out = attn @ w                  # (b, h, n, d)
    y = out.reshape(b, n, 1024) @ Wout + bout

Sharding: 8 cores = 2 batches x 4 head-groups (4 heads each). Each core
computes its 4 heads end-to-end including a partial output projection
(Wout rows for its heads); host sums the 4 partials per batch (the
"all-reduce" of the output projection) and adds bout.

Device kernel (per core) highlights:
  - softmax without max-subtraction (S*scale ~ N(0,1), exp is safe in fp32)
  - denominator comes free from ACT's accum_out during exp
  - post-softmax column mask folded into V (V' = (1-mask_j) * w_j)
  - E tiles are computed in [j, i] layout; since S is symmetric the same
    tiles serve the AV matmul (contract j on partitions) and the row-sum
    denominator (free-dim accum per partition row).
"""

import sys

if "/opt/trn_rl_repo" not in sys.path:
    sys.path.insert(0, "/opt/trn_rl_repo")

import numpy as np

import concourse.bass as bass
import concourse.mybir as mybir
from concourse import masks
from concourse.bass_utils import run_bass_kernel_spmd
from concourse.tile import TileContext

FP = mybir.dt.float32
I32 = mybir.dt.int32


def _split_multiwaits(bir_json: bytes) -> bytes:
    """This container's walrus supports a single sync wait per instruction
    (setupSyncWait: 'Too many sync wait commands', seen on the Tile tail
    Drain). Split any multi-wait instruction into a chain of single-wait
    EventSemaphore instructions (same engine, program order) followed by
    the original instruction keeping its last wait."""
    import json

    bir = json.loads(bir_json)
    changed = False
    for fn in bir.get("functions", []):
        for bb in fn.get("blocks", []):
            insts = bb.get("instructions")
            if insts is None:
                continue
            new_insts = []
            for ins in insts:
                si = ins.get("sync_info")
                waits = si.get("on_wait") if si else None
                if waits and len(waits) > 1:
                    changed = True
                    for wi, w in enumerate(waits[:-1]):
                        new_insts.append({
                            "name": f"{ins['name']}_w{wi}",
                            "opcode": "EventSemaphore",
                            "engine": ins["engine"],
                            "ins": [],
                            "outs": [],
                            "debug": ins.get("debug", 0),
                            "sync_info": {"on_wait": [w], "on_update": []},
                        })
                    si["on_wait"] = [waits[-1]]
                new_insts.append(ins)
            bb["instructions"] = new_insts
    if not changed:
        return bir_json
    return json.dumps(bir).encode()


def _install_bir_legalizer():
    from concourse import bass2jax, bass_utils

    if getattr(bass2jax, "_multiwait_legalizer_installed", False):
        return
    orig = bass_utils.compile_bir_kernel

    def wrapped(bir_json, tmpdir, neff_name="file.neff"):
        return orig(_split_multiwaits(bytes(bir_json)), tmpdir, neff_name)

    bass2jax.compile_bir_kernel = wrapped
    bass2jax._multiwait_legalizer_installed = True

N = 2048          # sequence length
DIM = 1024        # model dim
DH = 64           # head dim
HEADS_PER_CORE = 4
PAIRS = 2         # head pairs per core (2 heads = 128 partitions stacked)
EC = HEADS_PER_CORE * DH   # 256 local inner columns
KC = DIM // 128   # 8 contraction chunks for the projection
JC = N // 128     # 16 key chunks
SCALE = DH ** -0.5

_program_cache = {}


def build_program():
    nc = bass.Bass()

    xT = nc.declare_dram_parameter("xT", [DIM, N], FP, isOutput=False)
    wqkv = nc.declare_dram_parameter("wqkv", [DIM, EC], FP, isOutput=False)
    wout = nc.declare_dram_parameter("wout", [EC, DIM], FP, isOutput=False)
    mask_d = nc.declare_dram_parameter("mask", [N], I32, isOutput=False)
    y = nc.declare_dram_parameter("y", [N, DIM], FP, isOutput=True)

    EXPF = mybir.ActivationFunctionType.Exp

    with TileContext(nc) as tc:
        with (
            tc.tile_pool(name="const", bufs=1) as constp,
            tc.tile_pool(name="wts", bufs=1) as wts,
            tc.tile_pool(name="persist", bufs=1) as persist,
            tc.tile_pool(name="xin", bufs=3) as xin,
            tc.tile_pool(name="epool", bufs=4) as epool,
            tc.tile_pool(name="bsb", bufs=2) as bsb,
        ):
            # ---- constants / small inputs ----
            ident = constp.tile([128, 128], FP)
            masks.make_identity(nc, ident[:])
            ones64 = constp.tile([1, 64], FP)
            nc.vector.memset(ones64[:], 1.0)

            mask_i = constp.tile([16, 128], I32)
            nc.sync.dma_start(mask_i[:], mask_d.rearrange("(a b) -> a b", a=16))
            mask_f = constp.tile([16, 128], FP)
            # 1 - mask, cast int32 -> fp32
            nc.vector.tensor_scalar(
                out=mask_f[:], in0=mask_i[:], scalar1=-1.0, scalar2=1.0,
                op0=mybir.AluOpType.mult, op1=mybir.AluOpType.add,
            )

            # ---- weights ----
            wq_sb = wts.tile([128, KC, EC], FP)
            nc.sync.dma_start(wq_sb[:], wqkv.rearrange("(kc p) e -> p kc e", p=128))
            wout_sb = wts.tile([128, PAIRS, DIM], FP)
            nc.sync.dma_start(wout_sb[:], wout.rearrange("(pc p) m -> p pc m", p=128))

            # ---- persistent big tiles ----
            wT2 = persist.tile([128, PAIRS, N], FP)      # [d2, pair, i]
            v2 = persist.tile([128, PAIRS, JC, 128], FP)  # [j%128, pair, jc, d2]
            osT2 = persist.tile([128, PAIRS, N], FP)      # scaled attn out, [e, pair, i]
            maskc = persist.tile([128, JC], FP)           # (1-mask) in [j%128, jc]
            parts = persist.tile([128, HEADS_PER_CORE, JC, 2], FP)  # exp row-sum parts
            den = persist.tile([128, HEADS_PER_CORE, JC], FP)
            recip = persist.tile([128, HEADS_PER_CORE, JC], FP)
            recip_flat = persist.tile([1, HEADS_PER_CORE, N], FP)

            # ---- phase 1: projection  wT2[d2, i] = Wqkv_cols^T @ x^T ----
            with tc.tile_pool(name="ps_proj", bufs=1, space="PSUM") as ps_proj:
                proj_ps = [ps_proj.tile([128, 512], FP, name=f"proj{t}", tag=f"proj{t}")
                           for t in range(8)]
                for kc in range(KC):
                    xt = xin.tile([128, N], FP, name="xt")
                    nc.sync.dma_start(xt[:], xT[kc * 128:(kc + 1) * 128, :])
                    for pair in range(PAIRS):
                        for rb in range(4):
                            nc.tensor.matmul(
                                proj_ps[pair * 4 + rb][:],
                                wq_sb[:, kc, pair * 128:(pair + 1) * 128],
                                xt[:, rb * 512:(rb + 1) * 512],
                                start=(kc == 0), stop=(kc == KC - 1),
                            )
                for pair in range(PAIRS):
                    for rb in range(4):
                        nc.vector.tensor_copy(
                            wT2[:, pair, rb * 512:(rb + 1) * 512],
                            proj_ps[pair * 4 + rb][:],
                        )

            # ---- phase 2: transposes (mask layout + V') ----
            with tc.tile_pool(name="ps_tr", bufs=2, space="PSUM") as ps_tr:
                mt_ps = ps_tr.tile([128, 16], FP, tag="tr")
                nc.tensor.transpose(mt_ps[:], mask_f[:], ident[0:16, 0:16])
                nc.vector.tensor_copy(maskc[:], mt_ps[:])

                for pair in range(PAIRS):
                    for jc in range(JC):
                        tr_ps = ps_tr.tile([128, 128], FP, name="tr", tag="tr")
                        nc.tensor.transpose(
                            tr_ps[:], wT2[:, pair, jc * 128:(jc + 1) * 128], ident[:]
                        )
                        # V' = (1 - mask_j) * w_j, applied per partition (j)
                        nc.vector.tensor_scalar_mul(
                            v2[:, pair, jc, :], tr_ps[:], maskc[:, jc:jc + 1]
                        )

            # ---- phase 3: attention per pair ----
            with (
                tc.tile_pool(name="ps_s", bufs=2, space="PSUM") as ps_s,
                tc.tile_pool(name="ps_av", bufs=1, space="PSUM") as ps_av,
            ):
                for pair in range(PAIRS):
                    av_ps = ps_av.tile([128, N], FP, name="av", tag="av")
                    # Col-packed accumulation (two heads stacked on partitions)
                    # can't use start=True (one zero-region group per 2KB bank):
                    # pre-zero and accumulate with start=False throughout.
                    nc.vector.memset(av_ps[:], 0.0)
                    for jc in range(JC):
                        for hh in range(2):       # head within pair
                            h = pair * 2 + hh
                            p0 = hh * 64
                            for ibh in range(2):  # i half (1024 wide)
                                s_ps = ps_s.tile([128, 1024], FP, name="s", tag="s")
                                for sb in range(2):
                                    nc.tensor.matmul(
                                        s_ps[:, sb * 512:(sb + 1) * 512],
                                        wT2[p0:p0 + 64, pair, jc * 128:(jc + 1) * 128],
                                        wT2[p0:p0 + 64, pair,
                                            ibh * 1024 + sb * 512:ibh * 1024 + (sb + 1) * 512],
                                        start=True, stop=True,
                                        tile_position=(p0, 0),
                                    )
                                e_t = epool.tile([128, 1024], FP, name="e")
                                nc.scalar.activation(
                                    e_t[:], s_ps[:], EXPF, scale=SCALE,
                                    accum_out=parts[:, h, jc, ibh:ibh + 1],
                                )
                                for sb in range(2):
                                    ib = ibh * 2 + sb
                                    nc.tensor.matmul(
                                        av_ps[p0:p0 + 64, ib * 512:(ib + 1) * 512],
                                        v2[:, pair, jc, p0:p0 + 64],
                                        e_t[:, sb * 512:(sb + 1) * 512],
                                        start=False, stop=False,
                                        skip_group_check=True,
                                        tile_position=(0, p0),
                                    )

                    # ---- denominator -> reciprocal -> [1, i] layout ----
                    for hh in range(2):
                        h = pair * 2 + hh
                        nc.vector.tensor_tensor(
                            out=den[:, h, :], in0=parts[:, h, :, 0],
                            in1=parts[:, h, :, 1], op=mybir.AluOpType.add,
                        )
                        nc.vector.reciprocal(recip[:, h, :], den[:, h, :])
                        rt_ps = ps_s.tile([16, 128], FP, name="rt", tag="s")
                        nc.tensor.transpose(rt_ps[:], recip[:, h, :], ident[:])
                        rt_sb = bsb.tile([16, 128], FP, name="rts", tag="rts")
                        nc.vector.tensor_copy(rt_sb[:], rt_ps[:])
                        # [16, 128] -> [1, 2048] flatten: DMA pairs elements in
                        # AP iteration order (p-major), i.e. flat[jc*128+p].
                        nc.sync.dma_start(recip_flat[0:1, h, :], rt_sb[:])

                    # ---- broadcast 1/den along partitions and scale ----
                    for ibh in range(2):
                        b_ps = ps_s.tile([128, 1024], FP, name="bc", tag="s")
                        nc.vector.memset(b_ps[:], 0.0)
                        for hh in range(2):
                            h = pair * 2 + hh
                            for sb in range(2):
                                nc.tensor.matmul(
                                    b_ps[hh * 64:hh * 64 + 64, sb * 512:(sb + 1) * 512],
                                    ones64[:],
                                    recip_flat[0:1, h,
                                               ibh * 1024 + sb * 512:ibh * 1024 + (sb + 1) * 512],
                                    start=False, stop=False,
                                    skip_group_check=True,
                                    tile_position=(0, hh * 64),
                                )
                        b_sb = bsb.tile([128, 1024], FP, name="bcs")
                        nc.vector.tensor_copy(b_sb[:], b_ps[:])
                        nc.vector.tensor_tensor(
                            out=osT2[:, pair, ibh * 1024:(ibh + 1) * 1024],
                            in0=av_ps[:, ibh * 1024:(ibh + 1) * 1024],
                            in1=b_sb[:],
                            op=mybir.AluOpType.mult,
                        )

            # ---- phase 4: output projection (partial) ----
            with tc.tile_pool(name="ps_y", bufs=3, space="PSUM") as ps_y:
                for ic in range(16):
                    y_ps = ps_y.tile([128, 1024], FP, name="yp", tag="y")
                    for nb in range(2):
                        for pair in range(PAIRS):
                            nc.tensor.matmul(
                                y_ps[:, nb * 512:(nb + 1) * 512],
                                osT2[:, pair, ic * 128:(ic + 1) * 128],
                                wout_sb[:, pair, nb * 512:(nb + 1) * 512],
                                start=(pair == 0), stop=(pair == PAIRS - 1),
                            )
                    y_sb = bsb.tile([128, 1024], FP, name="ysb", tag="ysb", bufs=3)
                    if ic % 2 == 0:
                        nc.vector.tensor_copy(y_sb[:], y_ps[:])
                    else:
                        nc.scalar.copy(y_sb[:], y_ps[:])
                    nc.sync.dma_start(y[ic * 128:(ic + 1) * 128, :], y_sb[:])

    return nc


def get_program():
    if "nc" not in _program_cache:
        _program_cache["nc"] = build_program()
    return _program_cache["nc"]


def make_in_maps(x, mask, Wqkv, Wout):
    xT_b = [np.ascontiguousarray(x[b].T) for b in range(2)]
    in_maps = []
    for c in range(8):
        b, hg = c // 4, c % 4
        ec = slice(hg * EC, (hg + 1) * EC)
        in_maps.append({
            "xT": xT_b[b],
            "wqkv": np.ascontiguousarray(Wqkv[:, ec]),
            "wout": np.ascontiguousarray(Wout[ec, :]),
            "mask": np.ascontiguousarray(mask[b]),
        })
    return in_maps


def assemble(results, bout):
    y = np.stack([
        sum(results[b * 4 + g]["y"] for g in range(4)) for b in range(2)
    ])
    return (y + bout[None, None, :]).astype(np.float32)


def kernel(x, mask, Wqkv, Wout, bout):
    _install_bir_legalizer()
    nc = get_program()
    in_maps = make_in_maps(x, mask, Wqkv, Wout)
    res = run_bass_kernel_spmd(nc, in_maps, core_ids=list(range(8)))
    return assemble(res.results, bout)


if __name__ == "__main__":
    nc = build_program()
    print("program built OK")


# revision 14
# speedup vs baseline: 1.5089x; 1.5089x over previous
"""Trainium2 Bass kernel for CRATE-style subspace attention (nn_Attention_37091337568712).

Reference computation (fp32):
    w = x @ Wqkv                    # (b, n, 1024), shared q=k=v projection
    w -> (b, h=16, n, d=64)
    S = (w @ w^T) * d^-0.5          # per head, (b, h, n, n)
    attn = softmax(S, axis=-1) * (1 - mask[:, None, None, :])
    out = attn @ w                  # (b, h, n, d)
    y = out.reshape(b, n, 1024) @ Wout + bout

Sharding: 8 cores = 2 batches x 4 head-groups (4 heads each). Each core
computes its 4 heads end-to-end including a partial output projection
(Wout rows for its heads); host sums the 4 partials per batch (the
"all-reduce" of the output projection) and adds bout.

Device kernel (per core) highlights:
  - softmax without max-subtraction (S*scale ~ N(0,1), exp is safe in fp32)
  - denominator comes free from ACT's accum_out during exp
  - post-softmax column mask folded into V (V' = (1-mask_j) * w_j)
  - E tiles are computed in [j, i] layout; since S is symmetric the same
    tiles serve the AV matmul (contract j on partitions) and the row-sum
    denominator (free-dim accum per partition row).
"""

import sys

if "/opt/trn_rl_repo" not in sys.path:
    sys.path.insert(0, "/opt/trn_rl_repo")

import numpy as np

import concourse.bass as bass
import concourse.mybir as mybir
from concourse import masks
from concourse.bass_utils import run_bass_kernel_spmd
from concourse.tile import TileContext

FP = mybir.dt.float32
I32 = mybir.dt.int32
F32R = mybir.dt.float32r  # 4x faster PE path than fp32, ~fp32 accuracy


def _r(ap):
    return ap.bitcast(F32R)


def _split_multiwaits(bir_json: bytes) -> bytes:
    """This container's walrus supports a single sync wait per instruction
    (setupSyncWait: 'Too many sync wait commands', seen on the Tile tail
    Drain). Split any multi-wait instruction into a chain of single-wait
    EventSemaphore instructions (same engine, program order) followed by
    the original instruction keeping its last wait."""
    import json

    bir = json.loads(bir_json)
    changed = False
    for fn in bir.get("functions", []):
        for bb in fn.get("blocks", []):
            insts = bb.get("instructions")
            if insts is None:
                continue
            new_insts = []
            for ins in insts:
                si = ins.get("sync_info")
                waits = si.get("on_wait") if si else None
                if waits and len(waits) > 1:
                    changed = True
                    for wi, w in enumerate(waits[:-1]):
                        new_insts.append({
                            "name": f"{ins['name']}_w{wi}",
                            "opcode": "EventSemaphore",
                            "engine": ins["engine"],
                            "ins": [],
                            "outs": [],
                            "debug": ins.get("debug", 0),
                            "sync_info": {"on_wait": [w], "on_update": []},
                        })
                    si["on_wait"] = [waits[-1]]
                new_insts.append(ins)
            bb["instructions"] = new_insts
    if not changed:
        return bir_json
    return json.dumps(bir).encode()


def _install_bir_legalizer():
    from concourse import bass2jax, bass_utils

    if getattr(bass2jax, "_multiwait_legalizer_installed", False):
        return
    orig = bass_utils.compile_bir_kernel

    def wrapped(bir_json, tmpdir, neff_name="file.neff"):
        try:
            return orig(_split_multiwaits(bytes(bir_json)), tmpdir, neff_name)
        except BaseException as e:
            # XLA swallows python exceptions from the compile callback;
            # persist the real error for debugging.
            import subprocess, traceback
            try:
                with open("/tmp/bass_compile_err.txt", "w") as f:
                    traceback.print_exc(file=f)
                    ee = e
                    while ee is not None:
                        if isinstance(ee, subprocess.CalledProcessError):
                            out = ee.stdout or ""
                            if isinstance(out, bytes):
                                out = out.decode(errors="replace")
                            f.write("\n==WALRUS STDOUT (tail)==\n" + out[-12000:])
                        ee = ee.__cause__ or ee.__context__
            except Exception:
                pass
            raise

    bass2jax.compile_bir_kernel = wrapped
    bass2jax._multiwait_legalizer_installed = True

N = 2048          # sequence length
DIM = 1024        # model dim
DH = 64           # head dim
HEADS_PER_CORE = 4
PAIRS = 2         # head pairs per core (2 heads = 128 partitions stacked)
EC = HEADS_PER_CORE * DH   # 256 local inner columns
KC = DIM // 128   # 8 contraction chunks for the projection
JC = N // 128     # 16 key chunks
SCALE = DH ** -0.5

_program_cache = {}


def build_program():
    nc = bass.Bass()

    xT = nc.declare_dram_parameter("xT", [DIM, N], FP, isOutput=False)
    wqkv = nc.declare_dram_parameter("wqkv", [DIM, EC], FP, isOutput=False)
    wout = nc.declare_dram_parameter("wout", [EC, DIM], FP, isOutput=False)
    mask_d = nc.declare_dram_parameter("mask", [N], I32, isOutput=False)
    y = nc.declare_dram_parameter("y", [N, DIM], FP, isOutput=True)

    EXPF = mybir.ActivationFunctionType.Exp

    with TileContext(nc) as tc:
        with (
            tc.tile_pool(name="const", bufs=1) as constp,
            tc.tile_pool(name="wts", bufs=1) as wts,
            tc.tile_pool(name="persist", bufs=1) as persist,
            tc.tile_pool(name="xin", bufs=3) as xin,
            tc.tile_pool(name="epool", bufs=4) as epool,
            tc.tile_pool(name="bsb", bufs=2) as bsb,
        ):
            # ---- constants / small inputs ----
            ident = constp.tile([128, 128], FP)
            masks.make_identity(nc, ident[:])
            ones64 = constp.tile([1, 64], FP)
            nc.vector.memset(ones64[:], 1.0)

            mask_i = constp.tile([16, 128], I32)
            nc.sync.dma_start(mask_i[:], mask_d.rearrange("(a b) -> a b", a=16))
            mask_f = constp.tile([16, 128], FP)
            # 1 - mask, cast int32 -> fp32
            nc.vector.tensor_scalar(
                out=mask_f[:], in0=mask_i[:], scalar1=-1.0, scalar2=1.0,
                op0=mybir.AluOpType.mult, op1=mybir.AluOpType.add,
            )

            # ---- weights ----
            wq_sb = wts.tile([128, KC, EC], F32R)
            nc.sync.dma_start(wq_sb[:], _r(wqkv.rearrange("(kc p) e -> p kc e", p=128)))
            wout_sb = wts.tile([128, PAIRS, DIM], F32R)
            nc.sync.dma_start(wout_sb[:], _r(wout.rearrange("(pc p) m -> p pc m", p=128)))

            # ---- persistent big tiles ----
            wT2 = persist.tile([128, PAIRS, N], F32R)      # [d2, pair, i]
            v2 = persist.tile([128, PAIRS, JC, 128], F32R)  # [j%128, pair, jc, d2]
            osT2 = persist.tile([128, PAIRS, N], F32R)      # scaled attn out, [e, pair, i]
            raw2 = persist.tile([128, PAIRS, N], FP)        # unscaled attn out stash
            maskc = persist.tile([128, JC], FP)           # (1-mask) in [j%128, jc]
            parts = persist.tile([128, HEADS_PER_CORE, JC, 2], FP)  # exp row-sum parts
            den = persist.tile([128, HEADS_PER_CORE, JC], FP)
            recip = persist.tile([128, HEADS_PER_CORE, JC], FP)
            recip_flat = persist.tile([1, HEADS_PER_CORE, N], FP)

            # ---- phase 1: projection  wT2[d2, i] = Wqkv_cols^T @ x^T ----
            with tc.tile_pool(name="ps_proj", bufs=1, space="PSUM") as ps_proj:
                proj_ps = [ps_proj.tile([128, 512], FP, name=f"proj{t}", tag=f"proj{t}")
                           for t in range(8)]
                for kc in range(KC):
                    xt = xin.tile([128, N], F32R, name="xt")
                    nc.sync.dma_start(xt[:], _r(xT[kc * 128:(kc + 1) * 128, :]))
                    for pair in range(PAIRS):
                        for rb in range(4):
                            nc.tensor.matmul(
                                proj_ps[pair * 4 + rb][:],
                                wq_sb[:, kc, pair * 128:(pair + 1) * 128],
                                xt[:, rb * 512:(rb + 1) * 512],
                                start=(kc == 0), stop=(kc == KC - 1),
                            )
                for pair in range(PAIRS):
                    for rb in range(4):
                        nc.vector.tensor_copy(
                            wT2[:, pair, rb * 512:(rb + 1) * 512],
                            proj_ps[pair * 4 + rb][:],
                        )

            # ---- phase 2: transposes (mask layout + V') ----
            with tc.tile_pool(name="ps_tr", bufs=2, space="PSUM") as ps_tr:
                mt_ps = ps_tr.tile([128, 16], FP, tag="tr")
                nc.tensor.transpose(mt_ps[:], mask_f[:], ident[0:16, 0:16])
                nc.vector.tensor_copy(maskc[:], mt_ps[:])

                for pair in range(PAIRS):
                    for jc in range(JC):
                        tr_ps = ps_tr.tile([128, 128], FP, name="tr", tag="tr")
                        nc.tensor.transpose(
                            tr_ps[:], wT2[:, pair, jc * 128:(jc + 1) * 128].bitcast(FP), ident[:]
                        )
                        # V' = (1 - mask_j) * w_j, applied per partition (j)
                        nc.vector.tensor_scalar_mul(
                            v2[:, pair, jc, :], tr_ps[:], maskc[:, jc:jc + 1]
                        )

            # ---- phase 3: attention ----
            # For each (pair, i-half): S tiles for the two heads are computed
            # with row-disjoint PE tiles (rows 0:64 / 64:128, interleaved issue
            # order so they run concurrently), exp'd with free-dim accum_out
            # (softmax denominators), then AV accumulates per head into
            # [64, 1024] PSUM tiles (f32r matmul requires dst partition 0).
            with (
                tc.tile_pool(name="ps_s", bufs=2, space="PSUM") as ps_s,
                tc.tile_pool(name="ps_av", bufs=2, space="PSUM") as ps_av,
            ):
                for pair in range(PAIRS):
                    for ibh in range(2):
                        i0 = ibh * 1024
                        av_t = []
                        for hh in range(2):
                            av = ps_av.tile([64, 1024], FP, name=f"av{hh}", tag="av")
                            nc.vector.memset(av[:], 0.0)
                            av_t.append(av)
                        for jc in range(JC):
                            s_t = [ps_s.tile([128, 1024], FP, name=f"s{hh}", tag="s")
                                   for hh in range(2)]
                            for sb in range(2):
                                for hh in range(2):
                                    p0 = hh * 64
                                    nc.tensor.matmul(
                                        s_t[hh][:, sb * 512:(sb + 1) * 512],
                                        wT2[p0:p0 + 64, pair, jc * 128:(jc + 1) * 128],
                                        wT2[p0:p0 + 64, pair,
                                            i0 + sb * 512:i0 + (sb + 1) * 512],
                                        start=True, stop=True,
                                        tile_position=(p0, 0),
                                    )
                            e_t = []
                            for hh in range(2):
                                h = pair * 2 + hh
                                e = epool.tile([128, 1024], F32R, name=f"e{hh}", tag="e")
                                nc.scalar.activation(
                                    e[:], s_t[hh][:], EXPF, scale=SCALE,
                                    accum_out=parts[:, h, jc, ibh:ibh + 1],
                                )
                                e_t.append(e)
                            for sb in range(2):
                                for hh in range(2):
                                    nc.tensor.matmul(
                                        av_t[hh][:, sb * 512:(sb + 1) * 512],
                                        v2[:, pair, jc, hh * 64:hh * 64 + 64],
                                        e_t[hh][:, sb * 512:(sb + 1) * 512],
                                        start=False, stop=False,
                                        skip_group_check=True,
                                    )
                        for hh in range(2):
                            nc.vector.tensor_copy(
                                raw2[hh * 64:hh * 64 + 64, pair, i0:i0 + 1024],
                                av_t[hh][:],
                            )

                    # ---- denominator -> reciprocal -> [1, i] layout ----
                    for hh in range(2):
                        h = pair * 2 + hh
                        nc.vector.tensor_tensor(
                            out=den[:, h, :], in0=parts[:, h, :, 0],
                            in1=parts[:, h, :, 1], op=mybir.AluOpType.add,
                        )
                        nc.vector.reciprocal(recip[:, h, :], den[:, h, :])
                        rt_ps = ps_s.tile([16, 128], FP, name="rt", tag="s")
                        nc.tensor.transpose(rt_ps[:], recip[:, h, :], ident[:])
                        rt_sb = bsb.tile([16, 128], FP, name="rts", tag="rts")
                        nc.vector.tensor_copy(rt_sb[:], rt_ps[:])
                        # [16, 128] -> [1, 2048] flatten: DMA pairs elements in
                        # AP iteration order (p-major), i.e. flat[jc*128+p].
                        nc.sync.dma_start(recip_flat[0:1, h, :], rt_sb[:])

                    # ---- broadcast 1/den along partitions and scale ----
                    for ibh in range(2):
                        i0 = ibh * 1024
                        b_ps = ps_s.tile([128, 1024], FP, name="bc", tag="s")
                        nc.vector.memset(b_ps[:], 0.0)
                        for hh in range(2):
                            h = pair * 2 + hh
                            for sb in range(2):
                                nc.tensor.matmul(
                                    b_ps[hh * 64:hh * 64 + 64, sb * 512:(sb + 1) * 512],
                                    ones64[:],
                                    recip_flat[0:1, h, i0 + sb * 512:i0 + (sb + 1) * 512],
                                    start=False, stop=False,
                                    skip_group_check=True,
                                    tile_position=(0, hh * 64),
                                )
                        b_sb = bsb.tile([128, 1024], FP, name="bcs")
                        nc.vector.tensor_copy(b_sb[:], b_ps[:])
                        nc.vector.tensor_tensor(
                            out=osT2[:, pair, i0:i0 + 1024],
                            in0=raw2[:, pair, i0:i0 + 1024],
                            in1=b_sb[:],
                            op=mybir.AluOpType.mult,
                        )

            # ---- phase 4: output projection (partial) ----
            with tc.tile_pool(name="ps_y", bufs=3, space="PSUM") as ps_y:
                for ic in range(16):
                    y_ps = ps_y.tile([128, 1024], FP, name="yp", tag="y")
                    for nb in range(2):
                        for pair in range(PAIRS):
                            nc.tensor.matmul(
                                y_ps[:, nb * 512:(nb + 1) * 512],
                                osT2[:, pair, ic * 128:(ic + 1) * 128],
                                wout_sb[:, pair, nb * 512:(nb + 1) * 512],
                                start=(pair == 0), stop=(pair == PAIRS - 1),
                            )
                    y_sb = bsb.tile([128, 1024], FP, name="ysb", tag="ysb", bufs=3)
                    if ic % 2 == 0:
                        nc.vector.tensor_copy(y_sb[:], y_ps[:])
                    else:
                        nc.scalar.copy(y_sb[:], y_ps[:])
                    nc.sync.dma_start(y[ic * 128:(ic + 1) * 128, :], y_sb[:])

    return nc


def get_program():
    if "nc" not in _program_cache:
        _program_cache["nc"] = build_program()
    return _program_cache["nc"]


def make_in_maps(x, mask, Wqkv, Wout):
    xT_b = [np.ascontiguousarray(x[b].T) for b in range(2)]
    in_maps = []
    for c in range(8):
        b, hg = c // 4, c % 4
        ec = slice(hg * EC, (hg + 1) * EC)
        in_maps.append({
            "xT": xT_b[b],
            "wqkv": np.ascontiguousarray(Wqkv[:, ec]),
            "wout": np.ascontiguousarray(Wout[ec, :]),
            "mask": np.ascontiguousarray(mask[b]),
        })
    return in_maps


def assemble(results, bout):
    y = np.stack([
        sum(results[b * 4 + g]["y"] for g in range(4)) for b in range(2)
    ])
    return (y + bout[None, None, :]).astype(np.float32)


def kernel(x, mask, Wqkv, Wout, bout):
    _install_bir_legalizer()
    nc = get_program()
    in_maps = make_in_maps(x, mask, Wqkv, Wout)
    res = run_bass_kernel_spmd(nc, in_maps, core_ids=list(range(8)))
    return assemble(res.results, bout)


if __name__ == "__main__":
    nc = build_program()
    print("program built OK")


# revision 20
# speedup vs baseline: 1.8182x; 1.2050x over previous
"""Trainium2 Bass kernel for CRATE-style subspace attention (nn_Attention_37091337568712).

Reference computation (fp32):
    w = x @ Wqkv                    # (b, n, 1024), shared q=k=v projection
    w -> (b, h=16, n, d=64)
    S = (w @ w^T) * d^-0.5          # per head, (b, h, n, n)
    attn = softmax(S, axis=-1) * (1 - mask[:, None, None, :])
    out = attn @ w                  # (b, h, n, d)
    y = out.reshape(b, n, 1024) @ Wout + bout

Sharding: 8 cores = 2 batches x 4 head-groups (4 heads each). Each core
computes its 4 heads end-to-end including a partial output projection
(Wout rows for its heads); host sums the 4 partials per batch (the
"all-reduce" of the output projection) and adds bout.

Device kernel (per core) highlights:
  - softmax without max-subtraction (S*scale ~ N(0,1), exp is safe in fp32)
  - denominator comes free from ACT's accum_out during exp
  - post-softmax column mask folded into V (V' = (1-mask_j) * w_j)
  - E tiles are computed in [j, i] layout; since S is symmetric the same
    tiles serve the AV matmul (contract j on partitions) and the row-sum
    denominator (free-dim accum per partition row).
"""

import sys

if "/opt/trn_rl_repo" not in sys.path:
    sys.path.insert(0, "/opt/trn_rl_repo")

import numpy as np

import concourse.bass as bass
import concourse.mybir as mybir
from concourse import library_config, masks
from concourse.bass_utils import run_bass_kernel_spmd
from concourse.tile import TileContext

FP = mybir.dt.float32
I32 = mybir.dt.int32
F32R = mybir.dt.float32r  # 4x faster PE path than fp32, ~fp32 accuracy


def _r(ap):
    return ap.bitcast(F32R)


def _split_multiwaits(bir_json: bytes) -> bytes:
    """This container's walrus supports a single sync wait per instruction
    (setupSyncWait: 'Too many sync wait commands', seen on the Tile tail
    Drain). Split any multi-wait instruction into a chain of single-wait
    EventSemaphore instructions (same engine, program order) followed by
    the original instruction keeping its last wait."""
    import json

    bir = json.loads(bir_json)
    changed = False
    for fn in bir.get("functions", []):
        for bb in fn.get("blocks", []):
            insts = bb.get("instructions")
            if insts is None:
                continue
            new_insts = []
            for ins in insts:
                si = ins.get("sync_info")
                waits = si.get("on_wait") if si else None
                if waits and len(waits) > 1:
                    changed = True
                    for wi, w in enumerate(waits[:-1]):
                        new_insts.append({
                            "name": f"{ins['name']}_w{wi}",
                            "opcode": "EventSemaphore",
                            "engine": ins["engine"],
                            "ins": [],
                            "outs": [],
                            "debug": ins.get("debug", 0),
                            "sync_info": {"on_wait": [w], "on_update": []},
                        })
                    si["on_wait"] = [waits[-1]]
                new_insts.append(ins)
            bb["instructions"] = new_insts
    if not changed:
        return bir_json
    return json.dumps(bir).encode()


def _install_bir_legalizer():
    from concourse import bass2jax, bass_utils

    if getattr(bass2jax, "_multiwait_legalizer_installed", False):
        return
    orig = bass_utils.compile_bir_kernel

    def wrapped(bir_json, tmpdir, neff_name="file.neff"):
        try:
            return orig(_split_multiwaits(bytes(bir_json)), tmpdir, neff_name)
        except BaseException as e:
            # XLA swallows python exceptions from the compile callback;
            # persist the real error for debugging.
            import subprocess, traceback
            try:
                with open("/tmp/bass_compile_err.txt", "w") as f:
                    traceback.print_exc(file=f)
                    ee = e
                    while ee is not None:
                        if isinstance(ee, subprocess.CalledProcessError):
                            out = ee.stdout or ""
                            if isinstance(out, bytes):
                                out = out.decode(errors="replace")
                            f.write("\n==WALRUS STDOUT (tail)==\n" + out[-12000:])
                        ee = ee.__cause__ or ee.__context__
            except Exception:
                pass
            raise

    bass2jax.compile_bir_kernel = wrapped
    bass2jax._multiwait_legalizer_installed = True

N = 2048          # sequence length
DIM = 1024        # model dim
DH = 64           # head dim
HEADS_PER_CORE = 4
PAIRS = 2         # head pairs per core (2 heads = 128 partitions stacked)
EC = HEADS_PER_CORE * DH   # 256 local inner columns
KC = DIM // 128   # 8 contraction chunks for the projection
JC = N // 128     # 16 key chunks
SCALE = DH ** -0.5

_program_cache = {}


def build_program():
    nc = bass.Bass()

    xT = nc.declare_dram_parameter("xT", [DIM, N], FP, isOutput=False)
    wqkv = nc.declare_dram_parameter("wqkv", [DIM, EC], FP, isOutput=False)
    wout = nc.declare_dram_parameter("wout", [EC, DIM], FP, isOutput=False)
    mask_d = nc.declare_dram_parameter("mask", [N], I32, isOutput=False)
    y = nc.declare_dram_parameter("y", [N, DIM], FP, isOutput=True)

    EXPF = mybir.ActivationFunctionType.Exp

    with TileContext(nc) as tc:
        with (
            tc.tile_pool(name="const", bufs=1) as constp,
            tc.tile_pool(name="wts", bufs=1) as wts,
            tc.tile_pool(name="persist", bufs=1) as persist,
            tc.tile_pool(name="xin", bufs=3) as xin,
            tc.tile_pool(name="epool", bufs=4) as epool,
            tc.tile_pool(name="bsb", bufs=2) as bsb,
        ):
            # ---- constants / small inputs ----
            ident = constp.tile([128, 128], FP)
            masks.make_identity(nc, ident[:])
            ones128 = constp.tile([1, 128], FP)
            nc.vector.memset(ones128[:], 1.0)

            mask_i = constp.tile([16, 128], I32)
            nc.sync.dma_start(mask_i[:], mask_d.rearrange("(a b) -> a b", a=16))
            mask_f = constp.tile([16, 128], FP)
            # 1 - mask, cast int32 -> fp32
            nc.vector.tensor_scalar(
                out=mask_f[:], in0=mask_i[:], scalar1=-1.0, scalar2=1.0,
                op0=mybir.AluOpType.mult, op1=mybir.AluOpType.add,
            )

            # ---- weights ----
            wq_sb = wts.tile([128, KC, EC], F32R)
            nc.sync.dma_start(wq_sb[:], _r(wqkv.rearrange("(kc p) e -> p kc e", p=128)))
            wout_sb = wts.tile([128, PAIRS, DIM], F32R)
            nc.sync.dma_start(wout_sb[:], _r(wout.rearrange("(pc p) m -> p pc m", p=128)))

            # ---- persistent big tiles ----
            wT2 = persist.tile([128, PAIRS, N], F32R)      # [d2, pair, i]
            v2 = persist.tile([128, PAIRS, JC, 130], F32R)  # [j, pair, jc, d2+ones]
            osT2 = persist.tile([128, PAIRS, N], F32R)      # scaled attn out, [e, pair, i]
            raw2 = persist.tile([128, PAIRS, N], FP)        # unscaled attn out stash
            maskc = persist.tile([128, JC], FP)           # (1-mask) in [j%128, jc]
            # softmax denominators, spread [128, 8 per k] for a cheap batched
            # reciprocal (engine APs may only start at partition 0/32/64/96,
            # and DVE reciprocal costs ~8 cycles per element per lane)
            den_sp = persist.tile([128, 64], FP)
            recip_sp = persist.tile([128, 64], FP)

            # ---- phase 1: projection  wT2[d2, i] = Wqkv_cols^T @ x^T ----
            with tc.tile_pool(name="ps_proj", bufs=1, space="PSUM") as ps_proj:
                proj_ps = [ps_proj.tile([128, 512], FP, name=f"proj{t}", tag=f"proj{t}")
                           for t in range(8)]
                for kc in range(KC):
                    xt = xin.tile([128, N], F32R, name="xt")
                    nc.sync.dma_start(xt[:], _r(xT[kc * 128:(kc + 1) * 128, :]))
                    for pair in range(PAIRS):
                        for rb in range(4):
                            nc.tensor.matmul(
                                proj_ps[pair * 4 + rb][:],
                                wq_sb[:, kc, pair * 128:(pair + 1) * 128],
                                xt[:, rb * 512:(rb + 1) * 512],
                                start=(kc == 0), stop=(kc == KC - 1),
                            )
                for pair in range(PAIRS):
                    for rb in range(4):
                        nc.vector.tensor_copy(
                            wT2[:, pair, rb * 512:(rb + 1) * 512],
                            proj_ps[pair * 4 + rb][:],
                        )

            # ---- phase 2: transposes (mask layout + V') ----
            with tc.tile_pool(name="ps_tr", bufs=2, space="PSUM") as ps_tr:
                mt_ps = ps_tr.tile([128, 16], FP, tag="tr")
                nc.tensor.transpose(mt_ps[:], mask_f[:], ident[0:16, 0:16])
                nc.vector.tensor_copy(maskc[:], mt_ps[:])

                # unmasked ones columns (64 and 129): the AV matmul's M=65
                # weight includes them so row 64 of the AV accumulator becomes
                # the (unmasked) softmax denominator for free.
                nc.vector.memset(v2[:, :, :, 64:130:65].bitcast(FP), 1.0)
                for pair in range(PAIRS):
                    for jc in range(JC):
                        tr_ps = ps_tr.tile([128, 128], FP, name="tr", tag="tr")
                        nc.tensor.transpose(
                            tr_ps[:], wT2[:, pair, jc * 128:(jc + 1) * 128].bitcast(FP), ident[:]
                        )
                        # V' = (1 - mask_j) * w_j, applied per partition (j)
                        for hh in range(2):
                            nc.vector.tensor_scalar_mul(
                                v2[:, pair, jc, hh * 65:hh * 65 + 64],
                                tr_ps[:, hh * 64:hh * 64 + 64],
                                maskc[:, jc:jc + 1],
                            )

            # ---- phase 3: attention ----
            # For each (pair, i-half): S tiles for the two heads are computed
            # with row-disjoint PE tiles (rows 0:64 / 64:128, interleaved issue
            # order so they run concurrently), exp'd, then AV accumulates per
            # head into [65, 1024] PSUM tiles (f32r matmul requires dst
            # partition 0). The V' weights carry an unmasked ones column, so
            # row 64 of each AV accumulator is the softmax denominator.
            with (
                tc.tile_pool(name="ps_s", bufs=2, space="PSUM") as ps_s,
                tc.tile_pool(name="ps_av", bufs=2, space="PSUM") as ps_av,
            ):
                for pair in range(PAIRS):
                    for ibh in range(2):
                        i0 = ibh * 1024
                        av_t = []
                        for hh in range(2):
                            av = ps_av.tile([65, 1024], FP, name=f"av{hh}", tag="av")
                            nc.vector.memset(av[:], 0.0)
                            av_t.append(av)
                        for jc in range(JC):
                            s_t = [ps_s.tile([128, 1024], FP, name=f"s{hh}", tag="s")
                                   for hh in range(2)]
                            for sb in range(2):
                                for hh in range(2):
                                    p0 = hh * 64
                                    nc.tensor.matmul(
                                        s_t[hh][:, sb * 512:(sb + 1) * 512],
                                        wT2[p0:p0 + 64, pair, jc * 128:(jc + 1) * 128],
                                        wT2[p0:p0 + 64, pair,
                                            i0 + sb * 512:i0 + (sb + 1) * 512],
                                        start=True, stop=True,
                                        tile_position=(p0, 0),
                                    )
                            e_t = []
                            for hh in range(2):
                                e = epool.tile([128, 1024], F32R, name=f"e{hh}", tag="e")
                                nc.scalar.activation(e[:], s_t[hh][:], EXPF, scale=SCALE)
                                e_t.append(e)
                            for sb in range(2):
                                for hh in range(2):
                                    nc.tensor.matmul(
                                        av_t[hh][:, sb * 512:(sb + 1) * 512],
                                        v2[:, pair, jc, hh * 65:hh * 65 + 65],
                                        e_t[hh][:, sb * 512:(sb + 1) * 512],
                                        start=False, stop=False,
                                        skip_group_check=True,
                                    )
                        for hh in range(2):
                            k = (pair * 2 + ibh) * 2 + hh
                            nc.vector.tensor_copy(
                                raw2[hh * 64:hh * 64 + 64, pair, i0:i0 + 1024],
                                av_t[hh][0:64, :],
                            )
                            trow = bsb.tile([1, 1024], FP, name="trow", tag="trow")
                            nc.vector.tensor_copy(trow[:], av_t[hh][64:65, :])
                            # [1, 1024] -> [128, 8] spread (DMA iterates the
                            # out AP partition-major, i -> (i//8, i%8))
                            nc.sync.dma_start(den_sp[:, k * 8:(k + 1) * 8], trow[:])

                # ---- scale: osT2 = raw2 / den ----
                nc.vector.reciprocal(recip_sp[:], den_sp[:])
                for pair in range(PAIRS):
                    for ibh in range(2):
                        i0 = ibh * 1024
                        for hh in range(2):
                            k = (pair * 2 + ibh) * 2 + hh
                            rrow = bsb.tile([1, 1024], FP, name="rrow", tag="rrow")
                            nc.sync.dma_start(rrow[:], recip_sp[:, k * 8:(k + 1) * 8])
                            # broadcast 1/den along partitions via a K=1 ones
                            # matmul (plain fp32: f32r can't, and the gpsimd
                            # PartitionBroadcast ucode doesn't codegen here)
                            b_ps = ps_s.tile([128, 1024], FP, name="bps", tag="s")
                            for sb in range(2):
                                nc.tensor.matmul(
                                    b_ps[:, sb * 512:(sb + 1) * 512],
                                    ones128[:],
                                    rrow[0:1, sb * 512:(sb + 1) * 512],
                                    start=True, stop=True,
                                )
                            nc.vector.tensor_tensor(
                                out=osT2[hh * 64:hh * 64 + 64, pair, i0:i0 + 1024],
                                in0=raw2[hh * 64:hh * 64 + 64, pair, i0:i0 + 1024],
                                in1=b_ps[hh * 64:hh * 64 + 64, :],
                                op=mybir.AluOpType.mult,
                            )

            # ---- phase 4: output projection (partial) ----
            with tc.tile_pool(name="ps_y", bufs=3, space="PSUM") as ps_y:
                for ic in range(16):
                    y_ps = ps_y.tile([128, 1024], FP, name="yp", tag="y")
                    for nb in range(2):
                        for pair in range(PAIRS):
                            nc.tensor.matmul(
                                y_ps[:, nb * 512:(nb + 1) * 512],
                                osT2[:, pair, ic * 128:(ic + 1) * 128],
                                wout_sb[:, pair, nb * 512:(nb + 1) * 512],
                                start=(pair == 0), stop=(pair == PAIRS - 1),
                            )
                    y_sb = bsb.tile([128, 1024], FP, name="ysb", tag="ysb", bufs=3)
                    nc.vector.tensor_copy(y_sb[:], y_ps[:])
                    nc.sync.dma_start(y[ic * 128:(ic + 1) * 128, :], y_sb[:])

    return nc


def get_program():
    if "nc" not in _program_cache:
        _program_cache["nc"] = build_program()
    return _program_cache["nc"]


def make_in_maps(x, mask, Wqkv, Wout):
    xT_b = [np.ascontiguousarray(x[b].T) for b in range(2)]
    in_maps = []
    for c in range(8):
        b, hg = c // 4, c % 4
        ec = slice(hg * EC, (hg + 1) * EC)
        in_maps.append({
            "xT": xT_b[b],
            "wqkv": np.ascontiguousarray(Wqkv[:, ec]),
            "wout": np.ascontiguousarray(Wout[ec, :]),
            "mask": np.ascontiguousarray(mask[b]),
        })
    return in_maps


def assemble(results, bout):
    y = np.stack([
        sum(results[b * 4 + g]["y"] for g in range(4)) for b in range(2)
    ])
    return (y + bout[None, None, :]).astype(np.float32)


def kernel(x, mask, Wqkv, Wout, bout):
    _install_bir_legalizer()
    nc = get_program()
    in_maps = make_in_maps(x, mask, Wqkv, Wout)
    res = run_bass_kernel_spmd(nc, in_maps, core_ids=list(range(8)))
    return assemble(res.results, bout)


if __name__ == "__main__":
    nc = build_program()
    print("program built OK")
